# revision 95
# baseline (speedup 1.0000x reference)
"""TRN2 Bass kernel for GPT-style causal self-attention with RoPE (bf16).

Reference (B=2, S=2048, D=1024, H=16, dk=64):
  qkv = hidden @ c_attn_w + c_attn_b; rope(q), rope(k) via position_ids;
  out = softmax(causal(q k^T / 8)) v, merged heads, @ c_proj_w + c_proj_b.

Sharding across 8 NeuronCores: core c = 4*b + g handles batch b and head
group g (4 heads = 256 dims). Each core computes its full S x S attention
for its heads and a row-sliced c_proj partial; the host sums the 4
partials per batch.

Device pipeline per core (all matmuls bf16, fp32 PSUM accumulate):
  1. QKV weight-stationary: qkvT[do, s] = Wqkv_chunk^T @ hT directly in
     transposed layout (no PE transposes for q/k). Paired s-blocks
     accumulate in alternating PSUM banks. Rope is applied in the
     transposed layout: partition-swap via SBUF-SBUF DMAs (gpsimd
     queue) + 3 DVE ops against host-precomputed cosT/sinT tables.
     V is transposed back to natural [s, d] via PE transposes (slotted
     between QKV fills) with a ones column appended.
  2. Per head-pair, per 512-wide q chunk: scores^T via K=64 matmul pairs
     (two heads in PE quadrants); exp on ScalarE (scale=1/8, its only
     job); causal diagonal mask (0/1) multiplied on DVE post-exp; PV
     accumulates [v|1]^T P^T per head in alternating banks (row 64 =
     softmax denominators). PSUM is evicted to SBUF by DVE immediately;
     recip (DVE) + partition_broadcast (gpsimd, its only compute) +
     DVE multiply run off the PE critical path.
  3. Projection per 512-q chunk: projT = Wp^T @ attnT, DVE eviction
     (ScalarE Identity+bias in the with_bias variant), bf16 DMA out.

Scheduling for the in-order engine queues: the scores of the biggest
chunk (c=3, hp=0) are emitted inside stage 1, interleaved with the
remaining QKV fills, so ScalarE's exp pipeline is warm when attention
starts; st_ps lives on the right side of PSUM so it can coexist with
the stage-1 pools. In the attention phase, chunks are ordered big ones
first/last with small ones in the middle, and each chunk's PV pair is
emitted at the midpoint of the next chunk's scores, bounding live
exp'd tiles while giving exp a chunk of PE wall-time to drain.

Input DMA is split across the sync (hT) and gpsimd (weights, trig,
consts) queues to approach full HBM bandwidth at startup; outputs
alternate queues; the last chunk's normalize chain and half its
projection evictions use the by-then-idle ScalarE queue to shorten
the tail.

On top of that baseline: (1) the causal diagonal mask is applied as
ONE DVE multiply per tile against a duplicated [128,2,128] mask
constant; (2) each chunk's two normalize chains are interleaved (both
u-evictions+den DMAs, both reciprocals, both gpsimd broadcasts, then
the multiplies) so DVE and gpsimd overlap; (3) the gpsimd software-DGE
ring is kept short: q/k weights + wp/bp load via the sync queue and
the last two proj chunks' outputs go out on sync/scalar only, so
gpsimd's expensive ring drain (~5us) runs at ~158us fully overlapped
with compute instead of serializing after the last matmul.

Output per core: outT [1024, 2048] bf16 partial; host sums per batch.
Warm-up covers chunks (3,0)+(2,0) (28 tiles) with PV(3,0) consumed
at stage-1 end on the freed qkv PSUM bufs, so a full chunk+PV leave
the attention phase. Measured on trn2: 174.4-175.9 us across 7
samples (device DVFS noise +-1.5us; slow device phases sample 15-20%
higher), rel err ~4.5e-3 (gate 2e-2); f32r baseline was 332 us.
"""

from contextlib import ExitStack

import numpy as np
import ml_dtypes

import concourse.bacc as bacc
import concourse.tile as tile
import concourse.mybir as mybir
from concourse.bass_utils import run_bass_kernel_spmd

f32 = mybir.dt.float32
bf16 = mybir.dt.bfloat16
AF = mybir.ActivationFunctionType
ALU = mybir.AluOpType

S = 2048
D = 1024
HD = 256           # head dims per core (4 heads x 64)
SB = S // 128      # 16
KC = D // 128      # 8
NCH = S // 512     # 4
BF = ml_dtypes.bfloat16


def build_attention_nc(with_bias=False, num_devices=8):
    nc = bacc.Bacc("TRN2", target_bir_lowering=False, debug=False,
                   num_devices=num_devices)

    hT_d = nc.dram_tensor("hT", [D, S], bf16, kind="ExternalInput")
    wqkv_d = nc.dram_tensor("wqkv", [D, 768], bf16, kind="ExternalInput")
    cosT_d = nc.dram_tensor("cosT", [128, S], bf16, kind="ExternalInput")
    sinT_d = nc.dram_tensor("sinT", [128, S], bf16, kind="ExternalInput")
    wp_d = nc.dram_tensor("wp", [HD, D], bf16, kind="ExternalInput")
    bp_d = nc.dram_tensor("bp", [128, 8], f32, kind="ExternalInput")
    mask01_d = nc.dram_tensor("mask01", [128, 128], bf16, kind="ExternalInput")
    ones64_d = nc.dram_tensor("ones64", [128, 64], bf16, kind="ExternalInput")
    ident_d = nc.dram_tensor("ident", [128, 128], bf16, kind="ExternalInput")
    if with_bias:
        bqkv_d = nc.dram_tensor("bqkv", [1, 768], bf16, kind="ExternalInput")
        onesrow_d = nc.dram_tensor("ones_row", [1, 512], bf16,
                                   kind="ExternalInput")
    outT_d = nc.dram_tensor("outT", [D, S], bf16, kind="ExternalOutput")

    with tile.TileContext(nc) as tc, ExitStack() as top:
        const = top.enter_context(tc.tile_pool(name="const", bufs=1))
        ident = const.tile([128, 128], bf16, tag="ident")
        mask2 = const.tile([128, 2, 128], bf16, tag="mask2")
        bp_sb = const.tile([128, 8], f32, tag="bp")
        if with_bias:
            bqkv_sb = const.tile([1, 768], bf16, tag="bqkv")
            nc.sync.dma_start(bqkv_sb[:], bqkv_d.ap())
            ones_row = const.tile([1, 512], bf16, tag="ones_row")
            nc.sync.dma_start(ones_row[:], onesrow_d.ap())

        persist = top.enter_context(tc.tile_pool(name="persist", bufs=1))
        qT = [persist.tile([128, S], bf16, tag=f"qT{hp}", name=f"qT{hp}")
              for hp in range(2)]
        kT = [persist.tile([128, S], bf16, tag=f"kT{hp}", name=f"kT{hp}")
              for hp in range(2)]
        v_sb = persist.tile([128, SB, 4, 65], bf16, tag="v")
        ones64 = const.tile([128, 64], bf16, tag="ones64")

        wp_sb = persist.tile([128, 2, D], bf16, tag="wp")
        attnT = [persist.tile([128, S], bf16, tag=f"attnT{hp}",
                              name=f"attnT{hp}") for hp in range(2)]
        cosT = persist.tile([128, S], bf16, tag="cosT")
        sinT = persist.tile([128, S], bf16, tag="sinT")

        # attn-phase pools created first: st_ps (4 PSUM banks) coexists
        # with stage 1 (qkv_ps 2 + tr_ps 2) so scores of the biggest
        # chunk can be emitted during QKV to warm up the exp pipeline.
        st23 = top.enter_context(ExitStack())
        _lazy = {"on": False, "stack": None, "n": 0}

        def st_ps_tile():
            if "st" not in _lazy:
                _lazy["st"] = st23.enter_context(
                    tc.tile_pool(name="st_ps", bufs=2, space="PSUM",
                                 side="right"))
            _lazy["n"] += 1
            return _lazy["st"].tile([128, 2, 512], f32, tag="st_p",
                                    name=f"st_p{_lazy['n']}")
        pt_pool = st23.enter_context(tc.tile_pool(name="pt", bufs=26))
        u_pool = st23.enter_context(tc.tile_pool(name="u", bufs=3))
        nrm_pool = st23.enter_context(tc.tile_pool(name="nrm", bufs=2))
        pj_sb = st23.enter_context(tc.tile_pool(name="pj_sb", bufs=3))

        pts_map = {}

        def scores_tile(c, hp, kb):
            q0 = max(512 * c, 128 * kb)
            off = q0 - 512 * c
            st_p = st_ps_tile()
            for h2 in range(2):
                nc.tensor.matmul(
                    st_p[:, h2, off:512],
                    kT[hp][h2 * 64:(h2 + 1) * 64,
                           kb * 128:(kb + 1) * 128],
                    qT[hp][h2 * 64:(h2 + 1) * 64, q0:512 * (c + 1)],
                    start=True, stop=True, tile_position=(h2 * 64, 0))
            pt = pt_pool.tile([128, 2, 512], bf16, tag="pt")
            nc.scalar.activation(pt[:, :, off:512], st_p[:, :, off:512],
                                 AF.Exp, scale=0.125)
            if 128 * kb >= 512 * c:
                nc.vector.tensor_tensor(pt[:, :, off:off + 128],
                                        pt[:, :, off:off + 128],
                                        mask2[:], op=ALU.mult)
            pts_map.setdefault((c, hp), []).append((kb, off, pt))

        pv_state = {}

        def pv_part(c, hp, part, den_eng=None, bc_eng=None, pool=None):
            # half a PV burst: splitting the 3-7us matmul runs keeps
            # each burst near ScalarE's 2-tile exp backlog so the exp
            # stream doesn't starve mid-chunk.
            nkb = 4 * c + 4
            if part == 0:
                psum = pool if pool is not None else out_ps
                tg = "o_p" if pool is None else "qkv_p"
                pv_state[(c, hp)] = [psum.tile([128, 512], f32,
                                               tag=tg, name=f"o_p{j}")
                                     for j in range(2)]
                sel = pts_map[(c, hp)][0:nkb // 2]
            else:
                sel = pts_map[(c, hp)][nkb // 2:]
            o_ps = pv_state[(c, hp)]
            if den_eng is None:
                den_eng = nc.sync
            u0 = den0 = None
            for j, (kb, off, pt) in enumerate(sel):
                for h2 in range(2):
                    nc.tensor.matmul(
                        o_ps[h2][0:65, off:512],
                        v_sb[:, kb, 2 * hp + h2, :],
                        pt[:, h2, off:512],
                        start=(kb == 0), stop=(kb == nkb - 1))
                    if (part == 1 and h2 == 0 and j == len(sel) - 1):
                        # h2=0's accumulation is complete: start its
                        # normalize chain before h2=1's last matmul so
                        # the recip/broadcast latency is part-covered.
                        u0 = u_pool.tile([65, 512], f32, tag="u")
                        nc.vector.tensor_copy(u0[:], o_ps[0][0:65, :])
                        den0 = nrm_pool.tile([1, 512], f32, tag="den0")
                        den_eng.dma_start(den0[:], u0[64:65, :])
            if part == 1:
                del pts_map[(c, hp)]
                del pv_state[(c, hp)]
                pv_norm(c, hp, o_ps, den_eng, bc_eng, u0, den0)

        def pv_pair(c, hp, den_eng=None, bc_eng=None, pool=None):
            pv_part(c, hp, 0, den_eng, bc_eng, pool)
            pv_part(c, hp, 1, den_eng, bc_eng, pool)

        def pv_norm(c, hp, o_ps, den_eng, bc_eng, u0=None, d0=None):
            if den_eng is None:
                den_eng = nc.sync
            csl = slice(c * 512, (c + 1) * 512)
            # normalize: h2 chains interleaved so DVE/gpsimd overlap.
            us, dens, rcps = [], [], []
            for h2 in range(2):
                if h2 == 0 and u0 is not None:
                    us.append(u0)
                    dens.append(d0)
                    continue
                u = u_pool.tile([65, 512], f32, tag="u")
                nc.vector.tensor_copy(u[:], o_ps[h2][0:65, :])
                den0 = nrm_pool.tile([1, 512], f32, tag="den0")
                den_eng.dma_start(den0[:], u[64:65, :])
                us.append(u)
                dens.append(den0)
            for h2 in range(2):
                rcp0 = nrm_pool.tile([1, 512], f32, tag="rcp0")
                nc.vector.reciprocal_approx_fast(rcp0[:], dens[h2][:])
                rcps.append(rcp0)
            bcs = []
            if bc_eng is None:
                bc_eng = nc.gpsimd
            for h2 in range(2):
                bc = nrm_pool.tile([64, 512], f32, tag="bc")
                bc_eng.partition_broadcast(bc[:], rcps[h2][:])
                bcs.append(bc)
            nc.vector.tensor_tensor(attnT[hp][0:64, csl],
                                    us[0][0:64, :], bcs[0][:], op=ALU.mult)
            aTo = u_pool.tile([64, 512], bf16, tag="aTo")
            nc.vector.tensor_tensor(aTo[:], us[1][0:64, :], bcs[1][:],
                                    op=ALU.mult)
            den_eng.dma_start(attnT[hp][64:128, csl], aTo[:])

        # ============ stage 1: QKV + rope (transposed layout) ============
        with ExitStack() as st1, nc.named_scope("qkv"):
            _lazy["stack"] = st1
            hT_pool = st1.enter_context(tc.tile_pool(name="hT", bufs=1))
            w_pool = st1.enter_context(tc.tile_pool(name="w", bufs=1))
            vT_pool = st1.enter_context(tc.tile_pool(name="vT", bufs=1))
            tr_ps = st1.enter_context(
                tc.tile_pool(name="tr_ps", bufs=2, space="PSUM"))
            rope_pool = st1.enter_context(tc.tile_pool(name="rope", bufs=2))
            qcos_pool = st1.enter_context(tc.tile_pool(name="qcos", bufs=1))

            w_sb = [w_pool.tile([128, 768], bf16, tag=f"w{kc}", name=f"w{kc}")
                    for kc in range(KC)]
            hT_sb = [hT_pool.tile([128, S], bf16, tag=f"hT{kc}",
                                  name=f"hT{kc}") for kc in range(KC)]
            for kc in range(KC):
                nc.gpsimd.dma_start(w_sb[kc][:, 512:768],
                                    wqkv_d.ap()[kc * 128:(kc + 1) * 128,
                                                512:768])
                nc.sync.dma_start(hT_sb[kc][:, 0:512],
                                  hT_d.ap()[kc * 128:(kc + 1) * 128, 0:512])
            for kc in range(KC):
                nc.sync.dma_start(hT_sb[kc][:, 512:1024],
                                  hT_d.ap()[kc * 128:(kc + 1) * 128,
                                            512:1024])
            for kc in range(KC):
                nc.sync.dma_start(hT_sb[kc][:, 1024:S],
                                  hT_d.ap()[kc * 128:(kc + 1) * 128,
                                            1024:S])
            # q/k weights behind hT on sync (needed from ~35us); keeping
            # them off gpsimd shortens its software-DGE ring and drain.
            for kc in range(KC):
                nc.sync.dma_start(w_sb[kc][:, 0:512],
                                  wqkv_d.ap()[kc * 128:(kc + 1) * 128,
                                              0:512])
            # lower-priority loads, behind the critical w/hT chunks
            nc.gpsimd.dma_start(ident[:], ident_d.ap())
            nc.gpsimd.dma_start(cosT[:], cosT_d.ap())
            nc.gpsimd.dma_start(sinT[:], sinT_d.ap())
            nc.gpsimd.dma_start(ones64[:], ones64_d.ap())
            nc.scalar.copy(v_sb[:, :, :, 64],
                           ones64[:].rearrange("p (a b) -> p a b", a=SB))
            nc.gpsimd.dma_start(mask2[:, 0, :], mask01_d.ap())
            nc.gpsimd.dma_start(mask2[:, 1, :], mask01_d.ap())
            for kc2 in range(2):
                nc.sync.dma_start(wp_sb[:, kc2, :],
                                  wp_d.ap()[kc2 * 128:(kc2 + 1) * 128, :])
            nc.sync.dma_start(bp_sb[:], bp_d.ap())

            vT_sb = [vT_pool.tile([128, S], bf16, tag=f"vT{t}", name=f"vT{t}")
                     for t in range(2)]

            def qkv_fill_pair(pool, do, spair, qraw, inter=None):
                # s-blocks accumulate in alternating PSUM banks so
                # consecutive matmuls avoid the same-bank RMW penalty
                sls = [slice(sblk * 512, (sblk + 1) * 512)
                       for sblk in spair]
                ps = [pool.tile([128, 512], f32, tag="qkv_p",
                                name=f"qkv_p{j}")
                      for j, _ in enumerate(spair)]
                for kc in range(KC):
                    for p, sl in zip(ps, sls):
                        nc.tensor.matmul(
                            p[:], w_sb[kc][:, do * 128:(do + 1) * 128],
                            hT_sb[kc][:, sl], start=(kc == 0),
                            stop=(kc == KC - 1 and not with_bias))
                if with_bias:
                    for p in ps:
                        nc.tensor.matmul(
                            p[:], bqkv_sb[:, do * 128:(do + 1) * 128],
                            ones_row[:], start=False, stop=True)
                if inter is not None:
                    inter()
                for p, sl in zip(ps, sls):
                    if do >= 4:
                        nc.vector.tensor_copy(vT_sb[do - 4][:, sl], p[:])
                    else:
                        nc.vector.tensor_copy(qraw[:, sl], p[:])

            def rope_chunk(do, qraw):
                dest = (qT if do in (0, 1) else kT)[do % 2]
                qsw = rope_pool.tile([128, S], bf16, tag="qsw")
                for blk in range(4):
                    sp = (blk * 32 + 32) % 64 + 64 * (blk // 2)
                    nc.gpsimd.dma_start(qsw[blk * 32:blk * 32 + 32, :],
                                        qraw[sp:sp + 32, :])
                qcos = qcos_pool.tile([128, S], bf16, tag="qcos")
                nc.vector.tensor_tensor(qcos[:], qraw[:], cosT[:],
                                        op=ALU.mult)
                nc.vector.tensor_tensor(qsw[:], qsw[:], sinT[:],
                                        op=ALU.mult)
                nc.vector.tensor_tensor(dest[:], qcos[:], qsw[:],
                                        op=ALU.add)

            _tr = iter([(t, sb) for t in range(2) for sb in range(SB)])

            def transposes(n):
                for _ in range(n):
                    nxt = next(_tr, None)
                    if nxt is None:
                        return
                    t, sb = nxt
                    tp = tr_ps.tile([128, 128], bf16, tag="tp")
                    nc.tensor.matmul(
                        tp[:], vT_sb[t][:, sb * 128:(sb + 1) * 128],
                        ident[:], is_transpose=True,
                        start=True, stop=True)
                    nc.vector.tensor_copy(
                        v_sb[:, sb, 2 * t:2 * t + 2, 0:64],
                        tp[:].rearrange("p (h d) -> p h d", h=2))

            # phase A (4 psum banks): v chunks, then q/k of head-pair 0
            # with the V transposes slotted between matmul bursts
            with ExitStack() as stA:
                qkv_psA = stA.enter_context(
                    tc.tile_pool(name="qkv_psA", bufs=4, space="PSUM"))
                qkv_fill_pair(qkv_psA, 4, (0, 1), None)
                qkv_fill_pair(qkv_psA, 5, (0, 1), None)
                qkv_fill_pair(qkv_psA, 4, (2, 3), None)
                qkv_fill_pair(qkv_psA, 5, (2, 3), None)
                qraw0 = rope_pool.tile([128, S], bf16, tag="qraw")
                qkv_fill_pair(qkv_psA, 0, (0, 1), qraw0,
                              inter=lambda: transposes(8))
                qkv_fill_pair(qkv_psA, 0, (2, 3), qraw0,
                              inter=lambda: transposes(8))
                rope_chunk(0, qraw0)
                qraw2 = rope_pool.tile([128, S], bf16, tag="qraw")
                qkv_fill_pair(qkv_psA, 2, (0, 1), qraw2,
                              inter=lambda: transposes(8))
                qkv_fill_pair(qkv_psA, 2, (2, 3), qraw2,
                              inter=lambda: transposes(8))
                rope_chunk(2, qraw2)

            # phase B (2 psum banks, coexists with st_ps): q/k head-pair
            # 1 with scores of the biggest chunk interleaved (exp warm-up)
            qkv_psB = st1.enter_context(
                tc.tile_pool(name="qkv_psB", bufs=2, space="PSUM"))
            # warm-up covers chunks (3,0) AND (2,0): 28 tiles. The
            # leftover after the fills is interleaved with PV(3,0)
            # (qkv_psB is free by then), so a whole chunk+PV leave the
            # exp-paced attn phase for stage 1, where ScalarE idles.
            sc30 = iter([(3, 0, kb) for kb in range(16)] +
                        [(2, 0, kb) for kb in range(12)])

            def sc30_tiles():
                for _ in range(2):
                    t = next(sc30, None)
                    if t is not None:
                        scores_tile(*t)
            qraw1 = rope_pool.tile([128, S], bf16, tag="qraw")
            for sblk in range(NCH):
                qkv_fill_pair(qkv_psB, 1, (sblk,), qraw1,
                              inter=sc30_tiles)
            rope_chunk(1, qraw1)
            qraw3 = rope_pool.tile([128, S], bf16, tag="qraw")
            for sblk in range(NCH):
                qkv_fill_pair(qkv_psB, 3, (sblk,), qraw3,
                              inter=sc30_tiles)
            rope_chunk(3, qraw3)
            for j, t in enumerate(sc30):
                scores_tile(*t)
                if j == 3:
                    pv_pair(3, 0, pool=qkv_psB)

        # ============ stages 2+3 interleaved ============
        out_ps = st23.enter_context(
            tc.tile_pool(name="out_ps", bufs=2, space="PSUM"))
        pj_ps = st23.enter_context(
            tc.tile_pool(name="pj_ps", bufs=2, space="PSUM"))

        def proj_chunk(c, tail=False, late=False):
            csl = slice(c * 512, (c + 1) * 512)
            for dd in range(8):
                pp = pj_ps.tile([128, 512], f32, tag="pp")
                for kc2 in range(2):
                    nc.tensor.matmul(
                        pp[:], wp_sb[:, kc2, dd * 128:(dd + 1) * 128],
                        attnT[kc2][:, csl],
                        start=(kc2 == 0), stop=(kc2 == 1))
                po = pj_sb.tile([128, 512], bf16, tag="po")
                if with_bias or (tail and dd % 2 == 0):
                    nc.scalar.activation(po[:], pp[:], AF.Identity,
                                         bias=bp_sb[:, dd:dd + 1])
                else:
                    nc.vector.tensor_copy(po[:], pp[:])
                # late/tail chunks keep outputs off gpsimd so its
                # software-DGE ring finishes (and drains) early.
                if tail or late:
                    eng = nc.scalar if dd % 2 == 0 else nc.sync
                else:
                    eng = nc.gpsimd if dd % 2 == 0 else nc.sync
                eng.dma_start(
                    outT_d.ap()[dd * 128:(dd + 1) * 128, csl], po[:])

        with nc.named_scope("attn"):
            _lazy["on"] = True      # switch scores to the 3-buf st ring
            # Big chunks bracket the phase (their exp overlaps QKV, and
            # the tail keeps dense PE work); small chunks in the middle.
            # PV of the previous chunk is emitted mid-scores to bound the
            # number of live exp'd probability tiles.
            chunks = [(3, 0), (2, 0), (3, 1), (0, 0),
                      (0, 1), (1, 0), (1, 1), (2, 1)]
            done = {(3, 0)}
            for i in range(2, len(chunks)):
                c, hp = chunks[i]
                nkb = 4 * c + 4
                pc, php = chunks[i - 1]
                pending_proj = None
                for kb in range(nkb):
                    if kb == nkb // 2:
                        pv_pair(pc, php)
                        done.add((pc, php))
                        if (pc, 1 - php) in done:
                            pending_proj = pc
                    scores_tile(c, hp, kb)
                if pending_proj is not None:
                    proj_chunk(pending_proj, late=(i == len(chunks) - 1))
            pv_pair(2, 1, den_eng=nc.scalar)
            proj_chunk(2, tail=True)

    nc.finalize()
    return nc


def make_core_inputs(inputs, core, with_bias, _cache):
    """Host-side shard prep for one core. _cache is per-run (shared
    across the 4 cores of a batch)."""
    b, g = core // 4, core % 4

    if ("hT", b) not in _cache:
        hidden = np.asarray(inputs["hidden_states"], dtype=np.float32)
        _cache[("hT", b)] = np.ascontiguousarray(hidden[b].T).astype(BF)
    if ("trig", b) not in _cache:
        pos = np.asarray(inputs["position_ids"])
        inv_freq = (1.0 / (10000.0 **
                           (np.arange(0, 64, 2, dtype=np.float64) / 64.0)))
        # pattern[d, s] = pos[s] * invf[d % 32] over d in [0, 64)
        freqsT = inv_freq[:, None] * pos[b].astype(np.float64)[None, :]
        embT = np.concatenate([freqsT, freqsT], axis=0)     # [64, S]
        cosp = np.cos(embT)
        sinp = np.sin(embT)
        sinp[:32, :] *= -1.0
        _cache[("trig", b)] = (np.tile(cosp, (2, 1)).astype(BF),
                               np.tile(sinp, (2, 1)).astype(BF))

    caw = np.asarray(inputs["c_attn_w"], dtype=np.float32)
    cab = np.asarray(inputs["c_attn_b"], dtype=np.float32)
    cpw = np.asarray(inputs["c_proj_w"], dtype=np.float32)
    cpb = np.asarray(inputs["c_proj_b"], dtype=np.float32)

    cs = slice(g * HD, (g + 1) * HD)
    wqkv = np.concatenate(
        [caw[:, cs], caw[:, D + g * HD:D + (g + 1) * HD],
         caw[:, 2 * D + g * HD:2 * D + (g + 1) * HD]], axis=1)

    bp = (cpb if g == 0 else np.zeros_like(cpb)).reshape(8, 128).T.copy()

    r = np.arange(128)
    mask01 = (r[None, :] >= r[:, None]).astype(BF)
    cosT, sinT = _cache[("trig", b)]

    out = {
        "hT": _cache[("hT", b)],
        "wqkv": np.ascontiguousarray(wqkv).astype(BF),
        "cosT": cosT,
        "sinT": sinT,
        "wp": np.ascontiguousarray(cpw[cs, :]).astype(BF),
        "bp": np.ascontiguousarray(bp.astype(np.float32)),
        "mask01": mask01,
        "ones64": np.ones((128, 64), BF),
        "ident": np.eye(128).astype(BF),
    }
    if with_bias:
        bqkv = np.concatenate(
            [cab[cs], cab[D + g * HD:D + (g + 1) * HD],
             cab[2 * D + g * HD:2 * D + (g + 1) * HD]])[None, :]
        out["bqkv"] = bqkv.astype(BF)
        out["ones_row"] = np.ones((1, 512), BF)
    return out


_NC_CACHE = {}


def run(inputs, trace=False, **spmd_kwargs):
    """Shard, execute on 8 cores, unshard. Returns (output, BassKernelResults)."""
    with_bias = bool(np.any(np.asarray(inputs["c_attn_b"])) or
                     np.any(np.asarray(inputs["c_proj_b"])))
    if with_bias not in _NC_CACHE:
        _NC_CACHE[with_bias] = build_attention_nc(with_bias=with_bias,
                                                  num_devices=8)
    nc = _NC_CACHE[with_bias]
    prep_cache = {}
    in_maps = [make_core_inputs(inputs, c, with_bias, prep_cache)
               for c in range(8)]
    res = run_bass_kernel_spmd(nc, in_maps, core_ids=list(range(8)),
                               trace=trace, **spmd_kwargs)
    outs = []
    for b in range(2):
        acc = np.zeros((D, S), np.float32)
        for g in range(4):
            acc += res.results[b * 4 + g]["outT"].astype(np.float32)
        outs.append(acc.T)
    return np.stack(outs, axis=0), res


def kernel(**inputs) -> np.ndarray:
    out, _ = run(inputs, trace=False)
    return out



# revision 97
# speedup vs baseline: 1.1699x; 1.1699x over previous
"""TRN2 Bass kernel for GPT-style causal self-attention with RoPE (bf16).

Reference (B=2, S=2048, D=1024, H=16, dk=64):
  qkv = hidden @ c_attn_w + c_attn_b; rope(q), rope(k) via position_ids;
  out = softmax(causal(q k^T / 8)) v, merged heads, @ c_proj_w + c_proj_b.

Sharding across 8 NeuronCores: core c = 4*b + g handles batch b and head
group g (4 heads = 256 dims). Each core computes its full S x S attention
for its heads and a row-sliced c_proj partial; the host sums the 4
partials per batch.

Device pipeline per core (all matmuls bf16, fp32 PSUM accumulate):
  1. QKV weight-stationary: qkvT[do, s] = Wqkv_chunk^T @ hT directly in
     transposed layout (no PE transposes for q/k). Paired s-blocks
     accumulate in alternating PSUM banks. Rope is applied in the
     transposed layout: partition-swap via SBUF-SBUF DMAs (gpsimd
     queue) + 3 DVE ops against host-precomputed cosT/sinT tables.
     V is transposed back to natural [s, d] via PE transposes (slotted
     between QKV fills) with a ones column appended.
  2. Per head-pair, per 512-wide q chunk: scores^T via K=64 matmul pairs
     (two heads in PE quadrants); exp on ScalarE (scale=1/8, its only
     job); causal diagonal mask (0/1) multiplied on DVE post-exp; PV
     accumulates [v|1]^T P^T per head in alternating banks (row 64 =
     softmax denominators). PSUM is evicted to SBUF by DVE immediately;
     recip (DVE) + partition_broadcast (gpsimd, its only compute) +
     DVE multiply run off the PE critical path.
  3. Projection per 512-q chunk: projT = Wp^T @ attnT, DVE eviction
     (ScalarE Identity+bias in the with_bias variant), bf16 DMA out.

Scheduling for the in-order engine queues: the scores of the biggest
chunk (c=3, hp=0) are emitted inside stage 1, interleaved with the
remaining QKV fills, so ScalarE's exp pipeline is warm when attention
starts; st_ps lives on the right side of PSUM so it can coexist with
the stage-1 pools. In the attention phase, chunks are ordered big ones
first/last with small ones in the middle, and each chunk's PV pair is
emitted at the midpoint of the next chunk's scores, bounding live
exp'd tiles while giving exp a chunk of PE wall-time to drain.

Input DMA is split across the sync (hT) and gpsimd (weights, trig,
consts) queues to approach full HBM bandwidth at startup; outputs
alternate queues; the last chunk's normalize chain and half its
projection evictions use the by-then-idle ScalarE queue to shorten
the tail.

On top of that baseline: (1) the causal diagonal mask is applied as
ONE DVE multiply per tile against a duplicated [128,2,128] mask
constant; (2) each chunk's two normalize chains are interleaved (both
u-evictions+den DMAs, both reciprocals, both gpsimd broadcasts, then
the multiplies) so DVE and gpsimd overlap; (3) the gpsimd software-DGE
ring is kept short: q/k weights + wp/bp load via the sync queue and
the last two proj chunks' outputs go out on sync/scalar only, so
gpsimd's expensive ring drain (~5us) runs at ~158us fully overlapped
with compute instead of serializing after the last matmul.

Output per core: outT [1024, 2048] bf16 partial; host sums per batch.
Warm-up covers chunks (3,0)+(2,0) (28 tiles) with PV(3,0) consumed
at stage-1 end on the freed qkv PSUM bufs, so a full chunk+PV leave
the attention phase. Measured on trn2: 174.4-175.9 us across 7
samples (device DVFS noise +-1.5us; slow device phases sample 15-20%
higher), rel err ~4.5e-3 (gate 2e-2); f32r baseline was 332 us.
"""

from contextlib import ExitStack

import numpy as np
import ml_dtypes

import concourse.bacc as bacc
import concourse.tile as tile
import concourse.mybir as mybir
from concourse.bass_utils import run_bass_kernel_spmd

f32 = mybir.dt.float32
bf16 = mybir.dt.bfloat16
AF = mybir.ActivationFunctionType
ALU = mybir.AluOpType

S = 2048
D = 1024
HD = 256           # head dims per core (4 heads x 64)
SB = S // 128      # 16
KC = D // 128      # 8
NCH = S // 512     # 4
BF = ml_dtypes.bfloat16


def build_attention_nc(with_bias=False, num_devices=8):
    nc = bacc.Bacc("TRN2", target_bir_lowering=False, debug=False,
                   num_devices=num_devices)

    hT_d = nc.dram_tensor("hT", [D, S], bf16, kind="ExternalInput")
    wqkv_d = nc.dram_tensor("wqkv", [D, 768], bf16, kind="ExternalInput")
    cosT_d = nc.dram_tensor("cosT", [128, S], bf16, kind="ExternalInput")
    sinT_d = nc.dram_tensor("sinT", [128, S], bf16, kind="ExternalInput")
    wp_d = nc.dram_tensor("wp", [HD, D], bf16, kind="ExternalInput")
    bp_d = nc.dram_tensor("bp", [128, 8], f32, kind="ExternalInput")
    mask01_d = nc.dram_tensor("mask01", [128, 128], bf16, kind="ExternalInput")
    ones64_d = nc.dram_tensor("ones64", [128, 64], bf16, kind="ExternalInput")
    ident_d = nc.dram_tensor("ident", [128, 128], bf16, kind="ExternalInput")
    if with_bias:
        bqkv_d = nc.dram_tensor("bqkv", [1, 768], bf16, kind="ExternalInput")
        onesrow_d = nc.dram_tensor("ones_row", [1, 512], bf16,
                                   kind="ExternalInput")
    outT_d = nc.dram_tensor("outT", [D, S], bf16, kind="ExternalOutput")

    with tile.TileContext(nc) as tc, ExitStack() as top:
        const = top.enter_context(tc.tile_pool(name="const", bufs=1))
        ident = const.tile([128, 128], bf16, tag="ident")
        mask2 = const.tile([128, 2, 128], bf16, tag="mask2")
        bp_sb = const.tile([128, 8], f32, tag="bp")
        if with_bias:
            bqkv_sb = const.tile([1, 768], bf16, tag="bqkv")
            nc.sync.dma_start(bqkv_sb[:], bqkv_d.ap())
            ones_row = const.tile([1, 512], bf16, tag="ones_row")
            nc.sync.dma_start(ones_row[:], onesrow_d.ap())

        persist = top.enter_context(tc.tile_pool(name="persist", bufs=1))
        qT = [persist.tile([128, S], bf16, tag=f"qT{hp}", name=f"qT{hp}")
              for hp in range(2)]
        kT = [persist.tile([128, S], bf16, tag=f"kT{hp}", name=f"kT{hp}")
              for hp in range(2)]
        v_sb = persist.tile([128, SB, 4, 65], bf16, tag="v")
        ones64 = const.tile([128, 64], bf16, tag="ones64")

        wp_sb = persist.tile([128, 2, D], bf16, tag="wp")
        attnT = [persist.tile([128, S], bf16, tag=f"attnT{hp}",
                              name=f"attnT{hp}") for hp in range(2)]
        cosT = persist.tile([128, S], bf16, tag="cosT")
        sinT = persist.tile([128, S], bf16, tag="sinT")

        # attn-phase pools created first: st_ps (4 PSUM banks) coexists
        # with stage 1 (qkv_ps 2 + tr_ps 2) so scores of the biggest
        # chunk can be emitted during QKV to warm up the exp pipeline.
        st23 = top.enter_context(ExitStack())
        _lazy = {"on": False, "stack": None, "n": 0}

        def st_ps_tile():
            if "st" not in _lazy:
                _lazy["st"] = st23.enter_context(
                    tc.tile_pool(name="st_ps", bufs=2, space="PSUM",
                                 side="right"))
            _lazy["n"] += 1
            return _lazy["st"].tile([128, 2, 512], f32, tag="st_p",
                                    name=f"st_p{_lazy['n']}")
        pt_pool = st23.enter_context(tc.tile_pool(name="pt", bufs=26))
        u_pool = st23.enter_context(tc.tile_pool(name="u", bufs=3))
        nrm_pool = st23.enter_context(tc.tile_pool(name="nrm", bufs=2))
        pj_sb = st23.enter_context(tc.tile_pool(name="pj_sb", bufs=3))

        pts_map = {}

        def scores_tile(c, hp, kb):
            q0 = max(512 * c, 128 * kb)
            off = q0 - 512 * c
            st_p = st_ps_tile()
            for h2 in range(2):
                nc.tensor.matmul(
                    st_p[:, h2, off:512],
                    kT[hp][h2 * 64:(h2 + 1) * 64,
                           kb * 128:(kb + 1) * 128],
                    qT[hp][h2 * 64:(h2 + 1) * 64, q0:512 * (c + 1)],
                    start=True, stop=True, tile_position=(h2 * 64, 0))
            pt = pt_pool.tile([128, 2, 512], bf16, tag="pt")
            nc.scalar.activation(pt[:, :, off:512], st_p[:, :, off:512],
                                 AF.Exp, scale=0.125)
            if 128 * kb >= 512 * c:
                nc.vector.tensor_tensor(pt[:, :, off:off + 128],
                                        pt[:, :, off:off + 128],
                                        mask2[:], op=ALU.mult)
            pts_map.setdefault((c, hp), []).append((kb, off, pt))

        pv_state = {}

        def pv_part(c, hp, part, den_eng=None, bc_eng=None, pool=None):
            # half a PV burst: splitting the 3-7us matmul runs keeps
            # each burst near ScalarE's 2-tile exp backlog so the exp
            # stream doesn't starve mid-chunk.
            nkb = 4 * c + 4
            if part == 0:
                psum = pool if pool is not None else out_ps
                tg = "o_p" if pool is None else "qkv_p"
                pv_state[(c, hp)] = [psum.tile([128, 512], f32,
                                               tag=tg, name=f"o_p{j}")
                                     for j in range(2)]
                sel = pts_map[(c, hp)][0:nkb // 2]
            else:
                sel = pts_map[(c, hp)][nkb // 2:]
            o_ps = pv_state[(c, hp)]
            if den_eng is None:
                den_eng = nc.sync
            u0 = den0 = None
            for j, (kb, off, pt) in enumerate(sel):
                for h2 in range(2):
                    nc.tensor.matmul(
                        o_ps[h2][0:65, off:512],
                        v_sb[:, kb, 2 * hp + h2, :],
                        pt[:, h2, off:512],
                        start=(kb == 0), stop=(kb == nkb - 1))
                    if (part == 1 and h2 == 0 and j == len(sel) - 1):
                        # h2=0's accumulation is complete: start its
                        # normalize chain before h2=1's last matmul so
                        # the recip/broadcast latency is part-covered.
                        u0 = u_pool.tile([65, 512], f32, tag="u")
                        nc.vector.tensor_copy(u0[:], o_ps[0][0:65, :])
                        den0 = nrm_pool.tile([1, 512], f32, tag="den0")
                        den_eng.dma_start(den0[:], u0[64:65, :])
            if part == 1:
                del pts_map[(c, hp)]
                del pv_state[(c, hp)]
                pv_norm(c, hp, o_ps, den_eng, bc_eng, u0, den0)

        def pv_pair(c, hp, den_eng=None, bc_eng=None, pool=None):
            pv_part(c, hp, 0, den_eng, bc_eng, pool)
            pv_part(c, hp, 1, den_eng, bc_eng, pool)

        def pv_norm(c, hp, o_ps, den_eng, bc_eng, u0=None, d0=None):
            if den_eng is None:
                den_eng = nc.sync
            csl = slice(c * 512, (c + 1) * 512)
            # normalize: h2 chains interleaved so DVE/gpsimd overlap.
            us, dens, rcps = [], [], []
            for h2 in range(2):
                if h2 == 0 and u0 is not None:
                    us.append(u0)
                    dens.append(d0)
                    continue
                u = u_pool.tile([65, 512], f32, tag="u")
                nc.vector.tensor_copy(u[:], o_ps[h2][0:65, :])
                den0 = nrm_pool.tile([1, 512], f32, tag="den0")
                den_eng.dma_start(den0[:], u[64:65, :])
                us.append(u)
                dens.append(den0)
            for h2 in range(2):
                rcp0 = nrm_pool.tile([1, 512], f32, tag="rcp0")
                nc.vector.reciprocal_approx_fast(rcp0[:], dens[h2][:])
                rcps.append(rcp0)
            bcs = []
            if bc_eng is None:
                bc_eng = nc.gpsimd
            for h2 in range(2):
                bc = nrm_pool.tile([64, 512], f32, tag="bc")
                bc_eng.partition_broadcast(bc[:], rcps[h2][:])
                bcs.append(bc)
            nc.vector.tensor_tensor(attnT[hp][0:64, csl],
                                    us[0][0:64, :], bcs[0][:], op=ALU.mult)
            aTo = u_pool.tile([64, 512], bf16, tag="aTo")
            nc.vector.tensor_tensor(aTo[:], us[1][0:64, :], bcs[1][:],
                                    op=ALU.mult)
            den_eng.dma_start(attnT[hp][64:128, csl], aTo[:])

        # ============ stage 1: QKV + rope (transposed layout) ============
        with ExitStack() as st1, nc.named_scope("qkv"):
            _lazy["stack"] = st1
            hT_pool = st1.enter_context(tc.tile_pool(name="hT", bufs=1))
            w_pool = st1.enter_context(tc.tile_pool(name="w", bufs=1))
            vT_pool = st1.enter_context(tc.tile_pool(name="vT", bufs=1))
            tr_ps = st1.enter_context(
                tc.tile_pool(name="tr_ps", bufs=2, space="PSUM"))
            rope_pool = st1.enter_context(tc.tile_pool(name="rope", bufs=2))
            qcos_pool = st1.enter_context(tc.tile_pool(name="qcos", bufs=1))

            w_sb = [w_pool.tile([128, 768], bf16, tag=f"w{kc}", name=f"w{kc}")
                    for kc in range(KC)]
            hT_sb = [hT_pool.tile([128, S], bf16, tag=f"hT{kc}",
                                  name=f"hT{kc}") for kc in range(KC)]
            for kc in range(KC):
                nc.gpsimd.dma_start(w_sb[kc][:, 512:768],
                                    wqkv_d.ap()[kc * 128:(kc + 1) * 128,
                                                512:768])
                nc.sync.dma_start(hT_sb[kc][:, 0:512],
                                  hT_d.ap()[kc * 128:(kc + 1) * 128, 0:512])
            for kc in range(KC):
                nc.sync.dma_start(hT_sb[kc][:, 512:1024],
                                  hT_d.ap()[kc * 128:(kc + 1) * 128,
                                            512:1024])
            for kc in range(KC):
                nc.sync.dma_start(hT_sb[kc][:, 1024:S],
                                  hT_d.ap()[kc * 128:(kc + 1) * 128,
                                            1024:S])
            # q/k weights behind hT on sync (needed from ~35us); keeping
            # them off gpsimd shortens its software-DGE ring and drain.
            for kc in range(KC):
                nc.sync.dma_start(w_sb[kc][:, 0:512],
                                  wqkv_d.ap()[kc * 128:(kc + 1) * 128,
                                              0:512])
            # lower-priority loads, behind the critical w/hT chunks
            nc.gpsimd.dma_start(ident[:], ident_d.ap())
            nc.gpsimd.dma_start(cosT[:], cosT_d.ap())
            nc.gpsimd.dma_start(sinT[:], sinT_d.ap())
            nc.gpsimd.dma_start(ones64[:], ones64_d.ap())
            nc.scalar.copy(v_sb[:, :, :, 64],
                           ones64[:].rearrange("p (a b) -> p a b", a=SB))
            nc.gpsimd.dma_start(mask2[:, 0, :], mask01_d.ap())
            nc.gpsimd.dma_start(mask2[:, 1, :], mask01_d.ap())
            for kc2 in range(2):
                nc.sync.dma_start(wp_sb[:, kc2, :],
                                  wp_d.ap()[kc2 * 128:(kc2 + 1) * 128, :])
            nc.sync.dma_start(bp_sb[:], bp_d.ap())

            vT_sb = [vT_pool.tile([128, S], bf16, tag=f"vT{t}", name=f"vT{t}")
                     for t in range(2)]

            def qkv_fill_pair(pool, do, spair, qraw, inter=None):
                # s-blocks accumulate in alternating PSUM banks so
                # consecutive matmuls avoid the same-bank RMW penalty
                sls = [slice(sblk * 512, (sblk + 1) * 512)
                       for sblk in spair]
                ps = [pool.tile([128, 512], f32, tag="qkv_p",
                                name=f"qkv_p{j}")
                      for j, _ in enumerate(spair)]
                for kc in range(KC):
                    for p, sl in zip(ps, sls):
                        nc.tensor.matmul(
                            p[:], w_sb[kc][:, do * 128:(do + 1) * 128],
                            hT_sb[kc][:, sl], start=(kc == 0),
                            stop=(kc == KC - 1 and not with_bias))
                if with_bias:
                    for p in ps:
                        nc.tensor.matmul(
                            p[:], bqkv_sb[:, do * 128:(do + 1) * 128],
                            ones_row[:], start=False, stop=True)
                if inter is not None:
                    inter()
                for p, sl in zip(ps, sls):
                    if do >= 4:
                        nc.vector.tensor_copy(vT_sb[do - 4][:, sl], p[:])
                    else:
                        nc.vector.tensor_copy(qraw[:, sl], p[:])

            def rope_chunk(do, qraw):
                dest = (qT if do in (0, 1) else kT)[do % 2]
                qsw = rope_pool.tile([128, S], bf16, tag="qsw")
                for blk in range(4):
                    sp = (blk * 32 + 32) % 64 + 64 * (blk // 2)
                    nc.gpsimd.dma_start(qsw[blk * 32:blk * 32 + 32, :],
                                        qraw[sp:sp + 32, :])
                qcos = qcos_pool.tile([128, S], bf16, tag="qcos")
                nc.vector.tensor_tensor(qcos[:], qraw[:], cosT[:],
                                        op=ALU.mult)
                nc.vector.tensor_tensor(qsw[:], qsw[:], sinT[:],
                                        op=ALU.mult)
                nc.vector.tensor_tensor(dest[:], qcos[:], qsw[:],
                                        op=ALU.add)

            _tr = iter([(t, sb) for t in range(2) for sb in range(SB)])

            def transposes(n):
                for _ in range(n):
                    nxt = next(_tr, None)
                    if nxt is None:
                        return
                    t, sb = nxt
                    tp = tr_ps.tile([128, 128], bf16, tag="tp")
                    nc.tensor.matmul(
                        tp[:], vT_sb[t][:, sb * 128:(sb + 1) * 128],
                        ident[:], is_transpose=True,
                        start=True, stop=True)
                    nc.vector.tensor_copy(
                        v_sb[:, sb, 2 * t:2 * t + 2, 0:64],
                        tp[:].rearrange("p (h d) -> p h d", h=2))

            # phase A (4 psum banks): v chunks, then q/k of head-pair 0
            # with the V transposes slotted between matmul bursts
            with ExitStack() as stA:
                qkv_psA = stA.enter_context(
                    tc.tile_pool(name="qkv_psA", bufs=4, space="PSUM"))
                qkv_fill_pair(qkv_psA, 4, (0, 1), None)
                qkv_fill_pair(qkv_psA, 5, (0, 1), None)
                qkv_fill_pair(qkv_psA, 4, (2, 3), None)
                qkv_fill_pair(qkv_psA, 5, (2, 3), None)
                qraw0 = rope_pool.tile([128, S], bf16, tag="qraw")
                qkv_fill_pair(qkv_psA, 0, (0, 1), qraw0,
                              inter=lambda: transposes(8))
                qkv_fill_pair(qkv_psA, 0, (2, 3), qraw0,
                              inter=lambda: transposes(8))
                rope_chunk(0, qraw0)
                qraw2 = rope_pool.tile([128, S], bf16, tag="qraw")
                qkv_fill_pair(qkv_psA, 2, (0, 1), qraw2,
                              inter=lambda: transposes(8))
                qkv_fill_pair(qkv_psA, 2, (2, 3), qraw2,
                              inter=lambda: transposes(8))
                rope_chunk(2, qraw2)

            # phase B (2 psum banks, coexists with st_ps): q/k head-pair
            # 1 with scores of the biggest chunk interleaved (exp warm-up)
            qkv_psB = st1.enter_context(
                tc.tile_pool(name="qkv_psB", bufs=2, space="PSUM"))
            # warm-up covers chunks (3,0) AND (2,0): 28 tiles. The
            # leftover after the fills is interleaved with PV(3,0)
            # (qkv_psB is free by then), so a whole chunk+PV leave the
            # exp-paced attn phase for stage 1, where ScalarE idles.
            sc30 = iter([(3, 0, kb) for kb in range(16)] +
                        [(2, 0, kb) for kb in range(12)])

            def sc30_tiles():
                for _ in range(2):
                    t = next(sc30, None)
                    if t is not None:
                        scores_tile(*t)
            qraw1 = rope_pool.tile([128, S], bf16, tag="qraw")
            for sblk in range(NCH):
                qkv_fill_pair(qkv_psB, 1, (sblk,), qraw1,
                              inter=sc30_tiles)
            rope_chunk(1, qraw1)
            qraw3 = rope_pool.tile([128, S], bf16, tag="qraw")
            for sblk in range(NCH):
                qkv_fill_pair(qkv_psB, 3, (sblk,), qraw3,
                              inter=sc30_tiles)
            rope_chunk(3, qraw3)
            for j, t in enumerate(sc30):
                scores_tile(*t)
                if j == 3:
                    pv_pair(3, 0, pool=qkv_psB)

        # ============ stages 2+3 interleaved ============
        out_ps = st23.enter_context(
            tc.tile_pool(name="out_ps", bufs=2, space="PSUM"))
        pj_ps = st23.enter_context(
            tc.tile_pool(name="pj_ps", bufs=2, space="PSUM"))

        def proj_chunk(c, tail=False, late=False):
            csl = slice(c * 512, (c + 1) * 512)
            for dd in range(8):
                pp = pj_ps.tile([128, 512], f32, tag="pp")
                for kc2 in range(2):
                    nc.tensor.matmul(
                        pp[:], wp_sb[:, kc2, dd * 128:(dd + 1) * 128],
                        attnT[kc2][:, csl],
                        start=(kc2 == 0), stop=(kc2 == 1))
                po = pj_sb.tile([128, 512], bf16, tag="po")
                if with_bias or (tail and dd % 2 == 0):
                    nc.scalar.activation(po[:], pp[:], AF.Identity,
                                         bias=bp_sb[:, dd:dd + 1])
                else:
                    nc.vector.tensor_copy(po[:], pp[:])
                # late/tail chunks keep outputs off gpsimd so its
                # software-DGE ring finishes (and drains) early.
                if tail or late:
                    eng = nc.scalar if dd % 2 == 0 else nc.sync
                else:
                    eng = nc.gpsimd if dd % 2 == 0 else nc.sync
                eng.dma_start(
                    outT_d.ap()[dd * 128:(dd + 1) * 128, csl], po[:])

        with nc.named_scope("attn"):
            _lazy["on"] = True      # switch scores to the 3-buf st ring
            # Big chunks bracket the phase (their exp overlaps QKV, and
            # the tail keeps dense PE work); small chunks in the middle.
            # PV of the previous chunk is emitted mid-scores to bound the
            # number of live exp'd probability tiles.
            chunks = [(3, 0), (2, 0), (3, 1), (0, 0),
                      (0, 1), (1, 0), (1, 1), (2, 1)]
            done = {(3, 0)}
            for i in range(2, len(chunks)):
                c, hp = chunks[i]
                nkb = 4 * c + 4
                pc, php = chunks[i - 1]
                pending_proj = None
                for kb in range(nkb):
                    if kb == nkb // 2:
                        pv_pair(pc, php)
                        done.add((pc, php))
                        if (pc, 1 - php) in done:
                            pending_proj = pc
                    scores_tile(c, hp, kb)
                if pending_proj is not None:
                    proj_chunk(pending_proj, late=(i == len(chunks) - 1))
            pv_pair(2, 1, den_eng=nc.scalar)
            proj_chunk(2, tail=True)

    nc.finalize()
    return nc


def make_core_inputs(inputs, core, with_bias, _cache):
    """Host-side shard prep for one core. _cache is per-run (shared
    across the 4 cores of a batch)."""
    b, g = core // 4, core % 4

    if ("hT", b) not in _cache:
        hidden = np.asarray(inputs["hidden_states"], dtype=np.float32)
        _cache[("hT", b)] = np.ascontiguousarray(hidden[b].T).astype(BF)
    if ("trig", b) not in _cache:
        pos = np.asarray(inputs["position_ids"])
        inv_freq = (1.0 / (10000.0 **
                           (np.arange(0, 64, 2, dtype=np.float64) / 64.0)))
        # pattern[d, s] = pos[s] * invf[d % 32] over d in [0, 64)
        freqsT = inv_freq[:, None] * pos[b].astype(np.float64)[None, :]
        embT = np.concatenate([freqsT, freqsT], axis=0)     # [64, S]
        cosp = np.cos(embT)
        sinp = np.sin(embT)
        sinp[:32, :] *= -1.0
        _cache[("trig", b)] = (np.tile(cosp, (2, 1)).astype(BF),
                               np.tile(sinp, (2, 1)).astype(BF))

    caw = np.asarray(inputs["c_attn_w"], dtype=np.float32)
    cab = np.asarray(inputs["c_attn_b"], dtype=np.float32)
    cpw = np.asarray(inputs["c_proj_w"], dtype=np.float32)
    cpb = np.asarray(inputs["c_proj_b"], dtype=np.float32)

    cs = slice(g * HD, (g + 1) * HD)
    wqkv = np.concatenate(
        [caw[:, cs], caw[:, D + g * HD:D + (g + 1) * HD],
         caw[:, 2 * D + g * HD:2 * D + (g + 1) * HD]], axis=1)

    bp = (cpb if g == 0 else np.zeros_like(cpb)).reshape(8, 128).T.copy()

    r = np.arange(128)
    mask01 = (r[None, :] >= r[:, None]).astype(BF)
    cosT, sinT = _cache[("trig", b)]

    out = {
        "hT": _cache[("hT", b)],
        "wqkv": np.ascontiguousarray(wqkv).astype(BF),
        "cosT": cosT,
        "sinT": sinT,
        "wp": np.ascontiguousarray(cpw[cs, :]).astype(BF),
        "bp": np.ascontiguousarray(bp.astype(np.float32)),
        "mask01": mask01,
        "ones64": np.ones((128, 64), BF),
        "ident": np.eye(128).astype(BF),
    }
    if with_bias:
        bqkv = np.concatenate(
            [cab[cs], cab[D + g * HD:D + (g + 1) * HD],
             cab[2 * D + g * HD:2 * D + (g + 1) * HD]])[None, :]
        out["bqkv"] = bqkv.astype(BF)
        out["ones_row"] = np.ones((1, 512), BF)
    return out


_NC_CACHE = {}


def run(inputs, trace=False, **spmd_kwargs):
    """Shard, execute on 8 cores, unshard. Returns (output, BassKernelResults)."""
    with_bias = bool(np.any(np.asarray(inputs["c_attn_b"])) or
                     np.any(np.asarray(inputs["c_proj_b"])))
    if with_bias not in _NC_CACHE:
        _NC_CACHE[with_bias] = build_attention_nc(with_bias=with_bias,
                                                  num_devices=8)
    nc = _NC_CACHE[with_bias]
    prep_cache = {}
    in_maps = [make_core_inputs(inputs, c, with_bias, prep_cache)
               for c in range(8)]
    res = run_bass_kernel_spmd(nc, in_maps, core_ids=list(range(8)),
                               trace=trace, **spmd_kwargs)
    outs = []
    for b in range(2):
        acc = np.zeros((D, S), np.float32)
        for g in range(4):
            acc += res.results[b * 4 + g]["outT"].astype(np.float32)
        outs.append(acc.T)
    return np.stack(outs, axis=0), res


def kernel(**inputs) -> np.ndarray:
    out, _ = run(inputs, trace=False)
    return out



# revision 98
# speedup vs baseline: 1.1756x; 1.0049x over previous
"""TRN2 Bass kernel for GPT-style causal self-attention with RoPE (bf16).

Reference (B=2, S=2048, D=1024, H=16, dk=64):
  qkv = hidden @ c_attn_w + c_attn_b; rope(q), rope(k) via position_ids;
  out = softmax(causal(q k^T / 8)) v, merged heads, @ c_proj_w + c_proj_b.

Sharding across 8 NeuronCores: core c = 4*b + g handles batch b and head
group g (4 heads = 256 dims). Each core computes its full S x S attention
for its heads and a row-sliced c_proj partial; the host sums the 4
partials per batch.

Device pipeline per core (all matmuls bf16, fp32 PSUM accumulate):
  1. QKV weight-stationary: qkvT[do, s] = Wqkv_chunk^T @ hT directly in
     transposed layout (no PE transposes for q/k). Paired s-blocks
     accumulate in alternating PSUM banks. Rope is applied in the
     transposed layout: partition-swap via SBUF-SBUF DMAs (gpsimd
     queue) + 3 DVE ops against host-precomputed cosT/sinT tables.
     V is transposed back to natural [s, d] via PE transposes (slotted
     between QKV fills) with a ones column appended.
  2. Per head-pair, per 512-wide q chunk: scores^T via K=64 matmul pairs
     (two heads in PE quadrants); exp on ScalarE (scale=1/8, its only
     job); causal diagonal mask (0/1) multiplied on DVE post-exp; PV
     accumulates [v|1]^T P^T per head in alternating banks (row 64 =
     softmax denominators). PSUM is evicted to SBUF by DVE immediately;
     recip (DVE) + partition_broadcast (gpsimd, its only compute) +
     DVE multiply run off the PE critical path.
  3. Projection per 512-q chunk: projT = Wp^T @ attnT, DVE eviction
     (ScalarE Identity+bias in the with_bias variant), bf16 DMA out.

Scheduling for the in-order engine queues: the scores of the biggest
chunk (c=3, hp=0) are emitted inside stage 1, interleaved with the
remaining QKV fills, so ScalarE's exp pipeline is warm when attention
starts; st_ps lives on the right side of PSUM so it can coexist with
the stage-1 pools. In the attention phase, chunks are ordered big ones
first/last with small ones in the middle, and each chunk's PV pair is
emitted at the midpoint of the next chunk's scores, bounding live
exp'd tiles while giving exp a chunk of PE wall-time to drain.

Input DMA is split across the sync (hT) and gpsimd (weights, trig,
consts) queues to approach full HBM bandwidth at startup; outputs
alternate queues; the last chunk's normalize chain and half its
projection evictions use the by-then-idle ScalarE queue to shorten
the tail.

On top of that baseline: (1) the causal diagonal mask is applied as
ONE DVE multiply per tile against a duplicated [128,2,128] mask
constant; (2) each chunk's two normalize chains are interleaved (both
u-evictions+den DMAs, both reciprocals, both gpsimd broadcasts, then
the multiplies) so DVE and gpsimd overlap; (3) the gpsimd software-DGE
ring is kept short: q/k weights + wp/bp load via the sync queue and
the last two proj chunks' outputs go out on sync/scalar only, so
gpsimd's expensive ring drain (~5us) runs at ~158us fully overlapped
with compute instead of serializing after the last matmul.

Output per core: outT [1024, 2048] bf16 partial; host sums per batch.
Warm-up covers chunks (3,0)+(2,0) (28 tiles) with PV(3,0) consumed
at stage-1 end on the freed qkv PSUM bufs, so a full chunk+PV leave
the attention phase. Measured on trn2: 174.4-175.9 us across 7
samples (device DVFS noise +-1.5us; slow device phases sample 15-20%
higher), rel err ~4.5e-3 (gate 2e-2); f32r baseline was 332 us.
"""

from contextlib import ExitStack

import numpy as np
import ml_dtypes

import concourse.bacc as bacc
import concourse.tile as tile
import concourse.mybir as mybir
from concourse.bass_utils import run_bass_kernel_spmd

f32 = mybir.dt.float32
bf16 = mybir.dt.bfloat16
AF = mybir.ActivationFunctionType
ALU = mybir.AluOpType

S = 2048
D = 1024
HD = 256           # head dims per core (4 heads x 64)
SB = S // 128      # 16
KC = D // 128      # 8
NCH = S // 512     # 4
BF = ml_dtypes.bfloat16


def build_attention_nc(with_bias=False, num_devices=8):
    nc = bacc.Bacc("TRN2", target_bir_lowering=False, debug=False,
                   num_devices=num_devices)

    hT_d = nc.dram_tensor("hT", [D, S], bf16, kind="ExternalInput")
    wqkv_d = nc.dram_tensor("wqkv", [D, 768], bf16, kind="ExternalInput")
    cosT_d = nc.dram_tensor("cosT", [128, S], bf16, kind="ExternalInput")
    sinT_d = nc.dram_tensor("sinT", [128, S], bf16, kind="ExternalInput")
    wp_d = nc.dram_tensor("wp", [HD, D], bf16, kind="ExternalInput")
    bp_d = nc.dram_tensor("bp", [128, 8], f32, kind="ExternalInput")
    mask01_d = nc.dram_tensor("mask01", [128, 128], bf16, kind="ExternalInput")
    ones64_d = nc.dram_tensor("ones64", [128, 64], bf16, kind="ExternalInput")
    ident_d = nc.dram_tensor("ident", [128, 128], bf16, kind="ExternalInput")
    if with_bias:
        bqkv_d = nc.dram_tensor("bqkv", [1, 768], bf16, kind="ExternalInput")
        onesrow_d = nc.dram_tensor("ones_row", [1, 512], bf16,
                                   kind="ExternalInput")
    outT_d = nc.dram_tensor("outT", [D, S], bf16, kind="ExternalOutput")

    with tile.TileContext(nc) as tc, ExitStack() as top:
        const = top.enter_context(tc.tile_pool(name="const", bufs=1))
        ident = const.tile([128, 128], bf16, tag="ident")
        mask2 = const.tile([128, 2, 128], bf16, tag="mask2")
        bp_sb = const.tile([128, 8], f32, tag="bp")
        if with_bias:
            bqkv_sb = const.tile([1, 768], bf16, tag="bqkv")
            nc.sync.dma_start(bqkv_sb[:], bqkv_d.ap())
            ones_row = const.tile([1, 512], bf16, tag="ones_row")
            nc.sync.dma_start(ones_row[:], onesrow_d.ap())

        persist = top.enter_context(tc.tile_pool(name="persist", bufs=1))
        qT = [persist.tile([128, S], bf16, tag=f"qT{hp}", name=f"qT{hp}")
              for hp in range(2)]
        kT = [persist.tile([128, S], bf16, tag=f"kT{hp}", name=f"kT{hp}")
              for hp in range(2)]
        v_sb = persist.tile([128, SB, 4, 65], bf16, tag="v")
        ones64 = const.tile([128, 64], bf16, tag="ones64")

        wp_sb = persist.tile([128, 2, D], bf16, tag="wp")
        attnT = [persist.tile([128, S], bf16, tag=f"attnT{hp}",
                              name=f"attnT{hp}") for hp in range(2)]
        cosT = persist.tile([128, S], bf16, tag="cosT")
        sinT = persist.tile([128, S], bf16, tag="sinT")

        # attn-phase pools created first: st_ps (4 PSUM banks) coexists
        # with stage 1 (qkv_ps 2 + tr_ps 2) so scores of the biggest
        # chunk can be emitted during QKV to warm up the exp pipeline.
        st23 = top.enter_context(ExitStack())
        _lazy = {"on": False, "stack": None, "n": 0}

        def st_ps_tile():
            if "st" not in _lazy:
                _lazy["st"] = st23.enter_context(
                    tc.tile_pool(name="st_ps", bufs=2, space="PSUM",
                                 side="right"))
            _lazy["n"] += 1
            return _lazy["st"].tile([128, 2, 512], f32, tag="st_p",
                                    name=f"st_p{_lazy['n']}")
        pt_pool = st23.enter_context(tc.tile_pool(name="pt", bufs=26))
        u_pool = st23.enter_context(tc.tile_pool(name="u", bufs=3))
        nrm_pool = st23.enter_context(tc.tile_pool(name="nrm", bufs=2))
        pj_sb = st23.enter_context(tc.tile_pool(name="pj_sb", bufs=3))

        pts_map = {}

        def scores_tile(c, hp, kb):
            q0 = max(512 * c, 128 * kb)
            off = q0 - 512 * c
            st_p = st_ps_tile()
            for h2 in range(2):
                nc.tensor.matmul(
                    st_p[:, h2, off:512],
                    kT[hp][h2 * 64:(h2 + 1) * 64,
                           kb * 128:(kb + 1) * 128],
                    qT[hp][h2 * 64:(h2 + 1) * 64, q0:512 * (c + 1)],
                    start=True, stop=True, tile_position=(h2 * 64, 0))
            pt = pt_pool.tile([128, 2, 512], bf16, tag="pt")
            nc.scalar.activation(pt[:, :, off:512], st_p[:, :, off:512],
                                 AF.Exp, scale=0.125)
            if 128 * kb >= 512 * c:
                nc.vector.tensor_tensor(pt[:, :, off:off + 128],
                                        pt[:, :, off:off + 128],
                                        mask2[:], op=ALU.mult)
            pts_map.setdefault((c, hp), []).append((kb, off, pt))

        pv_state = {}

        def pv_part(c, hp, part, den_eng=None, bc_eng=None, pool=None):
            # half a PV burst: splitting the 3-7us matmul runs keeps
            # each burst near ScalarE's 2-tile exp backlog so the exp
            # stream doesn't starve mid-chunk.
            nkb = 4 * c + 4
            if part == 0:
                psum = pool if pool is not None else out_ps
                tg = "o_p" if pool is None else "qkv_p"
                pv_state[(c, hp)] = [psum.tile([128, 512], f32,
                                               tag=tg, name=f"o_p{j}")
                                     for j in range(2)]
                sel = pts_map[(c, hp)][0:nkb // 2]
            else:
                sel = pts_map[(c, hp)][nkb // 2:]
            o_ps = pv_state[(c, hp)]
            for (kb, off, pt) in sel:
                for h2 in range(2):
                    nc.tensor.matmul(
                        o_ps[h2][0:65, off:512],
                        v_sb[:, kb, 2 * hp + h2, :],
                        pt[:, h2, off:512],
                        start=(kb == 0), stop=(kb == nkb - 1))
            if part == 1:
                del pts_map[(c, hp)]
                del pv_state[(c, hp)]
                pv_norm(c, hp, o_ps, den_eng, bc_eng)

        def pv_pair(c, hp, den_eng=None, bc_eng=None, pool=None):
            pv_part(c, hp, 0, den_eng, bc_eng, pool)
            pv_part(c, hp, 1, den_eng, bc_eng, pool)

        def pv_norm(c, hp, o_ps, den_eng, bc_eng):
            if den_eng is None:
                den_eng = nc.sync
            csl = slice(c * 512, (c + 1) * 512)
            # normalize: h2 chains interleaved so DVE/gpsimd overlap.
            us, dens, rcps = [], [], []
            for h2 in range(2):
                u = u_pool.tile([65, 512], f32, tag="u")
                nc.vector.tensor_copy(u[:], o_ps[h2][0:65, :])
                den0 = nrm_pool.tile([1, 512], f32, tag="den0")
                den_eng.dma_start(den0[:], u[64:65, :])
                us.append(u)
                dens.append(den0)
            for h2 in range(2):
                rcp0 = nrm_pool.tile([1, 512], f32, tag="rcp0")
                nc.vector.reciprocal_approx_fast(rcp0[:], dens[h2][:])
                rcps.append(rcp0)
            bcs = []
            if bc_eng is None:
                bc_eng = nc.gpsimd
            for h2 in range(2):
                bc = nrm_pool.tile([64, 512], f32, tag="bc")
                bc_eng.partition_broadcast(bc[:], rcps[h2][:])
                bcs.append(bc)
            nc.vector.tensor_tensor(attnT[hp][0:64, csl],
                                    us[0][0:64, :], bcs[0][:], op=ALU.mult)
            aTo = u_pool.tile([64, 512], bf16, tag="aTo")
            nc.vector.tensor_tensor(aTo[:], us[1][0:64, :], bcs[1][:],
                                    op=ALU.mult)
            den_eng.dma_start(attnT[hp][64:128, csl], aTo[:])

        # ============ stage 1: QKV + rope (transposed layout) ============
        with ExitStack() as st1, nc.named_scope("qkv"):
            _lazy["stack"] = st1
            hT_pool = st1.enter_context(tc.tile_pool(name="hT", bufs=1))
            w_pool = st1.enter_context(tc.tile_pool(name="w", bufs=1))
            vT_pool = st1.enter_context(tc.tile_pool(name="vT", bufs=1))
            tr_ps = st1.enter_context(
                tc.tile_pool(name="tr_ps", bufs=2, space="PSUM"))
            rope_pool = st1.enter_context(tc.tile_pool(name="rope", bufs=2))
            qcos_pool = st1.enter_context(tc.tile_pool(name="qcos", bufs=1))

            w_sb = [w_pool.tile([128, 768], bf16, tag=f"w{kc}", name=f"w{kc}")
                    for kc in range(KC)]
            hT_sb = [hT_pool.tile([128, S], bf16, tag=f"hT{kc}",
                                  name=f"hT{kc}") for kc in range(KC)]
            for kc in range(KC):
                nc.gpsimd.dma_start(w_sb[kc][:, 512:768],
                                    wqkv_d.ap()[kc * 128:(kc + 1) * 128,
                                                512:768])
                nc.sync.dma_start(hT_sb[kc][:, 0:512],
                                  hT_d.ap()[kc * 128:(kc + 1) * 128, 0:512])
            for kc in range(KC):
                nc.sync.dma_start(hT_sb[kc][:, 512:1024],
                                  hT_d.ap()[kc * 128:(kc + 1) * 128,
                                            512:1024])
            for kc in range(KC):
                nc.sync.dma_start(hT_sb[kc][:, 1024:S],
                                  hT_d.ap()[kc * 128:(kc + 1) * 128,
                                            1024:S])
            # q/k weights behind hT on sync (needed from ~35us); keeping
            # them off gpsimd shortens its software-DGE ring and drain.
            for kc in range(KC):
                nc.sync.dma_start(w_sb[kc][:, 0:512],
                                  wqkv_d.ap()[kc * 128:(kc + 1) * 128,
                                              0:512])
            # lower-priority loads, behind the critical w/hT chunks
            nc.gpsimd.dma_start(ident[:], ident_d.ap())
            nc.gpsimd.dma_start(cosT[:], cosT_d.ap())
            nc.gpsimd.dma_start(sinT[:], sinT_d.ap())
            nc.gpsimd.dma_start(ones64[:], ones64_d.ap())
            nc.scalar.copy(v_sb[:, :, :, 64],
                           ones64[:].rearrange("p (a b) -> p a b", a=SB))
            nc.gpsimd.dma_start(mask2[:, 0, :], mask01_d.ap())
            nc.gpsimd.dma_start(mask2[:, 1, :], mask01_d.ap())
            for kc2 in range(2):
                nc.sync.dma_start(wp_sb[:, kc2, :],
                                  wp_d.ap()[kc2 * 128:(kc2 + 1) * 128, :])
            nc.sync.dma_start(bp_sb[:], bp_d.ap())

            vT_sb = [vT_pool.tile([128, S], bf16, tag=f"vT{t}", name=f"vT{t}")
                     for t in range(2)]

            def qkv_fill_pair(pool, do, spair, qraw, inter=None):
                # s-blocks accumulate in alternating PSUM banks so
                # consecutive matmuls avoid the same-bank RMW penalty
                sls = [slice(sblk * 512, (sblk + 1) * 512)
                       for sblk in spair]
                ps = [pool.tile([128, 512], f32, tag="qkv_p",
                                name=f"qkv_p{j}")
                      for j, _ in enumerate(spair)]
                for kc in range(KC):
                    for p, sl in zip(ps, sls):
                        nc.tensor.matmul(
                            p[:], w_sb[kc][:, do * 128:(do + 1) * 128],
                            hT_sb[kc][:, sl], start=(kc == 0),
                            stop=(kc == KC - 1 and not with_bias))
                if with_bias:
                    for p in ps:
                        nc.tensor.matmul(
                            p[:], bqkv_sb[:, do * 128:(do + 1) * 128],
                            ones_row[:], start=False, stop=True)
                if inter is not None:
                    inter()
                for p, sl in zip(ps, sls):
                    if do >= 4:
                        nc.vector.tensor_copy(vT_sb[do - 4][:, sl], p[:])
                    else:
                        nc.vector.tensor_copy(qraw[:, sl], p[:])

            def rope_chunk(do, qraw):
                dest = (qT if do in (0, 1) else kT)[do % 2]
                qsw = rope_pool.tile([128, S], bf16, tag="qsw")
                for blk in range(4):
                    sp = (blk * 32 + 32) % 64 + 64 * (blk // 2)
                    nc.gpsimd.dma_start(qsw[blk * 32:blk * 32 + 32, :],
                                        qraw[sp:sp + 32, :])
                qcos = qcos_pool.tile([128, S], bf16, tag="qcos")
                nc.vector.tensor_tensor(qcos[:], qraw[:], cosT[:],
                                        op=ALU.mult)
                nc.vector.tensor_tensor(qsw[:], qsw[:], sinT[:],
                                        op=ALU.mult)
                nc.vector.tensor_tensor(dest[:], qcos[:], qsw[:],
                                        op=ALU.add)

            _tr = iter([(t, sb) for t in range(2) for sb in range(SB)])

            def transposes(n):
                for _ in range(n):
                    nxt = next(_tr, None)
                    if nxt is None:
                        return
                    t, sb = nxt
                    tp = tr_ps.tile([128, 128], bf16, tag="tp")
                    nc.tensor.matmul(
                        tp[:], vT_sb[t][:, sb * 128:(sb + 1) * 128],
                        ident[:], is_transpose=True,
                        start=True, stop=True)
                    nc.vector.tensor_copy(
                        v_sb[:, sb, 2 * t:2 * t + 2, 0:64],
                        tp[:].rearrange("p (h d) -> p h d", h=2))

            # phase A (4 psum banks): v chunks, then q/k of head-pair 0
            # with the V transposes slotted between matmul bursts
            with ExitStack() as stA:
                qkv_psA = stA.enter_context(
                    tc.tile_pool(name="qkv_psA", bufs=4, space="PSUM"))
                qkv_fill_pair(qkv_psA, 4, (0, 1), None)
                qkv_fill_pair(qkv_psA, 5, (0, 1), None)
                qkv_fill_pair(qkv_psA, 4, (2, 3), None)
                qkv_fill_pair(qkv_psA, 5, (2, 3), None)
                qraw0 = rope_pool.tile([128, S], bf16, tag="qraw")
                qkv_fill_pair(qkv_psA, 0, (0, 1), qraw0,
                              inter=lambda: transposes(8))
                qkv_fill_pair(qkv_psA, 0, (2, 3), qraw0,
                              inter=lambda: transposes(8))
                rope_chunk(0, qraw0)
                qraw2 = rope_pool.tile([128, S], bf16, tag="qraw")
                qkv_fill_pair(qkv_psA, 2, (0, 1), qraw2,
                              inter=lambda: transposes(8))
                qkv_fill_pair(qkv_psA, 2, (2, 3), qraw2,
                              inter=lambda: transposes(8))
                rope_chunk(2, qraw2)

            # phase B (2 psum banks, coexists with st_ps): q/k head-pair
            # 1 with scores of the biggest chunk interleaved (exp warm-up)
            qkv_psB = st1.enter_context(
                tc.tile_pool(name="qkv_psB", bufs=2, space="PSUM"))
            # warm-up covers chunks (3,0) AND (2,0): 28 tiles. The
            # leftover after the fills is interleaved with PV(3,0)
            # (qkv_psB is free by then), so a whole chunk+PV leave the
            # exp-paced attn phase for stage 1, where ScalarE idles.
            sc30 = iter([(3, 0, kb) for kb in range(16)] +
                        [(2, 0, kb) for kb in range(12)])

            def sc30_tiles():
                for _ in range(2):
                    t = next(sc30, None)
                    if t is not None:
                        scores_tile(*t)
            qraw1 = rope_pool.tile([128, S], bf16, tag="qraw")
            for sblk in range(NCH):
                qkv_fill_pair(qkv_psB, 1, (sblk,), qraw1,
                              inter=sc30_tiles)
            rope_chunk(1, qraw1)
            qraw3 = rope_pool.tile([128, S], bf16, tag="qraw")
            for sblk in range(NCH):
                qkv_fill_pair(qkv_psB, 3, (sblk,), qraw3,
                              inter=sc30_tiles)
            rope_chunk(3, qraw3)
            for j, t in enumerate(sc30):
                scores_tile(*t)
                if j == 3:
                    pv_pair(3, 0, pool=qkv_psB)

        # ============ stages 2+3 interleaved ============
        out_ps = st23.enter_context(
            tc.tile_pool(name="out_ps", bufs=2, space="PSUM"))
        pj_ps = st23.enter_context(
            tc.tile_pool(name="pj_ps", bufs=2, space="PSUM"))

        def proj_chunk(c, tail=False, late=False):
            csl = slice(c * 512, (c + 1) * 512)
            for dd in range(8):
                pp = pj_ps.tile([128, 512], f32, tag="pp")
                for kc2 in range(2):
                    nc.tensor.matmul(
                        pp[:], wp_sb[:, kc2, dd * 128:(dd + 1) * 128],
                        attnT[kc2][:, csl],
                        start=(kc2 == 0), stop=(kc2 == 1))
                po = pj_sb.tile([128, 512], bf16, tag="po")
                if with_bias or (tail and dd % 2 == 0):
                    nc.scalar.activation(po[:], pp[:], AF.Identity,
                                         bias=bp_sb[:, dd:dd + 1])
                else:
                    nc.vector.tensor_copy(po[:], pp[:])
                # late/tail chunks keep outputs off gpsimd so its
                # software-DGE ring finishes (and drains) early.
                if tail or late:
                    eng = nc.scalar if dd % 2 == 0 else nc.sync
                else:
                    eng = nc.gpsimd if dd % 2 == 0 else nc.sync
                eng.dma_start(
                    outT_d.ap()[dd * 128:(dd + 1) * 128, csl], po[:])

        with nc.named_scope("attn"):
            _lazy["on"] = True      # switch scores to the 3-buf st ring
            # Big chunks bracket the phase (their exp overlaps QKV, and
            # the tail keeps dense PE work); small chunks in the middle.
            # PV of the previous chunk is emitted mid-scores to bound the
            # number of live exp'd probability tiles.
            chunks = [(3, 0), (2, 0), (3, 1), (0, 0),
                      (0, 1), (1, 0), (1, 1), (2, 1)]
            done = {(3, 0)}
            for i in range(2, len(chunks)):
                c, hp = chunks[i]
                nkb = 4 * c + 4
                pc, php = chunks[i - 1]
                pending_proj = None
                for kb in range(nkb):
                    if kb == nkb // 2:
                        pv_pair(pc, php)
                        done.add((pc, php))
                        if (pc, 1 - php) in done:
                            pending_proj = pc
                    scores_tile(c, hp, kb)
                if pending_proj is not None:
                    proj_chunk(pending_proj, late=(i == len(chunks) - 1))
            pv_pair(2, 1, den_eng=nc.scalar)
            proj_chunk(2, tail=True)

    nc.finalize()
    return nc


def make_core_inputs(inputs, core, with_bias, _cache):
    """Host-side shard prep for one core. _cache is per-run (shared
    across the 4 cores of a batch)."""
    b, g = core // 4, core % 4

    if ("hT", b) not in _cache:
        hidden = np.asarray(inputs["hidden_states"], dtype=np.float32)
        _cache[("hT", b)] = np.ascontiguousarray(hidden[b].T).astype(BF)
    if ("trig", b) not in _cache:
        pos = np.asarray(inputs["position_ids"])
        inv_freq = (1.0 / (10000.0 **
                           (np.arange(0, 64, 2, dtype=np.float64) / 64.0)))
        # pattern[d, s] = pos[s] * invf[d % 32] over d in [0, 64)
        freqsT = inv_freq[:, None] * pos[b].astype(np.float64)[None, :]
        embT = np.concatenate([freqsT, freqsT], axis=0)     # [64, S]
        cosp = np.cos(embT)
        sinp = np.sin(embT)
        sinp[:32, :] *= -1.0
        _cache[("trig", b)] = (np.tile(cosp, (2, 1)).astype(BF),
                               np.tile(sinp, (2, 1)).astype(BF))

    caw = np.asarray(inputs["c_attn_w"], dtype=np.float32)
    cab = np.asarray(inputs["c_attn_b"], dtype=np.float32)
    cpw = np.asarray(inputs["c_proj_w"], dtype=np.float32)
    cpb = np.asarray(inputs["c_proj_b"], dtype=np.float32)

    cs = slice(g * HD, (g + 1) * HD)
    wqkv = np.concatenate(
        [caw[:, cs], caw[:, D + g * HD:D + (g + 1) * HD],
         caw[:, 2 * D + g * HD:2 * D + (g + 1) * HD]], axis=1)

    bp = (cpb if g == 0 else np.zeros_like(cpb)).reshape(8, 128).T.copy()

    r = np.arange(128)
    mask01 = (r[None, :] >= r[:, None]).astype(BF)
    cosT, sinT = _cache[("trig", b)]

    out = {
        "hT": _cache[("hT", b)],
        "wqkv": np.ascontiguousarray(wqkv).astype(BF),
        "cosT": cosT,
        "sinT": sinT,
        "wp": np.ascontiguousarray(cpw[cs, :]).astype(BF),
        "bp": np.ascontiguousarray(bp.astype(np.float32)),
        "mask01": mask01,
        "ones64": np.ones((128, 64), BF),
        "ident": np.eye(128).astype(BF),
    }
    if with_bias:
        bqkv = np.concatenate(
            [cab[cs], cab[D + g * HD:D + (g + 1) * HD],
             cab[2 * D + g * HD:2 * D + (g + 1) * HD]])[None, :]
        out["bqkv"] = bqkv.astype(BF)
        out["ones_row"] = np.ones((1, 512), BF)
    return out


_NC_CACHE = {}


def run(inputs, trace=False, **spmd_kwargs):
    """Shard, execute on 8 cores, unshard. Returns (output, BassKernelResults)."""
    with_bias = bool(np.any(np.asarray(inputs["c_attn_b"])) or
                     np.any(np.asarray(inputs["c_proj_b"])))
    if with_bias not in _NC_CACHE:
        _NC_CACHE[with_bias] = build_attention_nc(with_bias=with_bias,
                                                  num_devices=8)
    nc = _NC_CACHE[with_bias]
    prep_cache = {}
    in_maps = [make_core_inputs(inputs, c, with_bias, prep_cache)
               for c in range(8)]
    res = run_bass_kernel_spmd(nc, in_maps, core_ids=list(range(8)),
                               trace=trace, **spmd_kwargs)
    outs = []
    for b in range(2):
        acc = np.zeros((D, S), np.float32)
        for g in range(4):
            acc += res.results[b * 4 + g]["outT"].astype(np.float32)
        outs.append(acc.T)
    return np.stack(outs, axis=0), res


def kernel(**inputs) -> np.ndarray:
    out, _ = run(inputs, trace=False)
    return out

